# revision 57
# baseline (speedup 1.0000x reference)
"""2-layer GCN on 8 Trainium2 NeuronCores — aggregate-then-project.

Nodes are range-sharded across 8 cores (dst parallel). Both GCN layers are
computed as: gather source rows from a bf16 feature table (dma_gather with
biased signed-int16 indices), segment-sum via one-hot bf16 matmuls into PSUM,
then per-dst-tile projection:

  layer1 table = dinv*x (host-precomputed, full -> no collective needed)
      aggx[d]  = sum_{s->d} table1[s]          (self-loops applied densely)
      t2[d]    = dinv[d] * relu(dinv[d]*(aggx[d] @ W1) + b1)
  AllGather t2 -> table2  [PADN_ALL, 128]: payload in cols 0:64, junk in
      64:128 that only ever reaches unread PSUM partitions -- both layers'
      stationary G operands stay 128-wide (FWL-hidden LDWEIGHTS) and one
      edge-metadata set (idx16/dstloc/one-hot layout) serves both layers.
      out[d]   = dinv[d]*(agg2[d] @ W2) + b2

Per group of 4 dst tiles, 4 sub-gathers land in per-queue G sub-tiles
(independent WAR release); consume runs all aggregation matmuls of the
group first (4 PSUM banks) and the per-tile projection tails afterwards,
so the next group's gathers are released as early as possible.
"""
import os
import sys

sys.path.insert(0, "/opt/trn_rl_repo")

import numpy as np
import ml_dtypes

import concourse.bass as bass
import concourse.bacc as bacc
import concourse.tile as tile
import concourse.mybir as mybir
from concourse import bass_utils
from concourse.library_config import mlp

N_CORES = 8
N_NODES = 100000
D_IN, D_H, D_OUT = 128, 64, 64
NSHARD = N_NODES // N_CORES          # 12500
TILE = 128
NT = (NSHARD + TILE - 1) // TILE     # 98
PADN = NT * TILE                     # 12544
PADN_ALL = N_CORES * PADN            # 100352
B0_ROWS = PADN_ALL // 2              # 50176 bucket split for int16 idx bias
BIAS = (32768, B0_ROWS + 32768)
N_BUCKET = 2
GROUP = 4
GROUPS = [(t, min(t + GROUP, NT)) for t in range(0, NT, GROUP)]
N_GROUP = len(GROUPS)                # 25 (24x4 tiles + 1x2)

LAST_RESULT = None


def _pack_meta(core, t_id, dloc, idx_val_raw, bucket):
    """Slot layout + per-core idx16/dstloc (shared by both edge passes)."""
    key = (core * NT + t_id) * N_BUCKET + bucket
    order = np.argsort(key, kind="stable")
    key_s = key[order]
    idx_s = idx_val_raw[order]
    dloc_s = dloc[order]

    ngroups = N_CORES * NT * N_BUCKET
    counts = np.bincount(key_s, minlength=ngroups).reshape(N_CORES, NT, N_BUCKET)
    nb = -(-counts.max(axis=0) // 128)                  # [NT, N_BUCKET] ceil
    nb = np.maximum(nb, 1)
    # each sub-gather ends at (tmid-1, b) or (t1-1, b); those (t,b) must end
    # with >=1 pad slot on every core (the gather ucode trims trailing
    # negative idxs, which would otherwise drop real edges).
    for (t0, t1) in GROUPS:
        tmid = (t0 + t1) // 2
        for tf in (tmid - 1, t1 - 1):
            for b in range(N_BUCKET):
                if (counts[:, tf, b] == nb[tf, b] * 128).any():
                    nb[tf, b] += 1

    # slot layout: per group of GROUP tiles: all b0 chunks (tile-major), then
    # all b1 chunks -> one contiguous gather dst region per (group, bucket).
    chunk_col = np.zeros((NT, N_BUCKET), np.int64)
    grp_nc = np.zeros(N_GROUP, np.int64)
    grp_base = np.zeros(N_GROUP, np.int64)
    grp_b_off = np.zeros((N_GROUP, N_BUCKET + 1), np.int64)
    pos = 0
    for g, (t0, t1) in enumerate(GROUPS):
        grp_base[g] = pos
        for b in range(N_BUCKET):
            grp_b_off[g, b] = pos - grp_base[g]
            for t in range(t0, t1):
                chunk_col[t, b] = pos
                pos += nb[t, b]
        grp_nc[g] = pos - grp_base[g]
        grp_b_off[g, N_BUCKET] = grp_nc[g]
    CHC = pos
    IDXC16 = CHC * 8

    grp_start = np.zeros(ngroups + 1, np.int64)
    np.cumsum(counts.reshape(-1), out=grp_start[1:])
    rank = np.arange(key_s.shape[0], dtype=np.int64) - grp_start[key_s]

    core_s = key_s // (NT * N_BUCKET)
    tb = key_s % (NT * N_BUCKET)
    t_s = tb // N_BUCKET
    b_s = tb % N_BUCKET

    slot = chunk_col[t_s, b_s] * 128 + rank
    ccol = slot // 128
    cpart = slot % 128

    assert idx_s.min() >= -32768 and idx_s.max() <= 32767

    idx16_16 = np.zeros((N_CORES, 16, IDXC16), np.int16)
    idx16_16[core_s, slot % 16, slot // 16] = idx_s.astype(np.int16)
    idx16 = np.tile(idx16_16, (1, 8, 1))                # [cores, 128, IDXC16]

    dstloc = np.full((N_CORES, 128, CHC), 512.0, np.float32)
    dstloc[core_s, cpart, ccol] = dloc_s.astype(np.float32)
    dstloc = dstloc.astype(ml_dtypes.bfloat16)

    return dict(nb=nb, chunk_col=chunk_col, grp_nc=grp_nc, grp_base=grp_base,
                grp_b_off=grp_b_off, CHC=CHC, IDXC16=IDXC16,
                idx16=idx16, dstloc=dstloc)


def _host_prep(x, edge_index):
    src = np.asarray(edge_index[0], dtype=np.int64)
    dst = np.asarray(edge_index[1], dtype=np.int64)
    n = N_NODES

    deg = np.bincount(dst, minlength=n).astype(np.float64) + 1.0
    dinv = (1.0 / np.sqrt(deg)).astype(np.float32)

    core_d = dst // NSHARD
    drem = dst % NSHARD
    t_id = drem // TILE
    dloc = drem % TILE

    # shared edge metadata: both layers' tables are [PADN_ALL, 128] with row
    # gsrc (padded global node index) -> one idx16/dstloc set serves both.
    gsrc = (src // NSHARD) * PADN + (src % NSHARD)
    bkt = (gsrc >= B0_ROWS).astype(np.int64)
    idxv = gsrc - np.where(bkt == 0, BIAS[0], BIAS[1])
    meta = _pack_meta(core_d, t_id, dloc, idxv, bkt)

    dinv_cols = np.zeros((N_CORES, 128, NT), np.float32)
    node_grid = (
        np.arange(N_CORES)[:, None, None] * NSHARD
        + np.arange(NT)[None, None, :] * TILE
        + np.arange(128)[None, :, None]
    )
    local = np.arange(NT)[None, None, :] * TILE + np.arange(128)[None, :, None]
    valid = np.broadcast_to(local < NSHARD, node_grid.shape)
    dinv_cols[:] = np.where(valid, dinv[np.where(valid, node_grid, 0)], 0.0)

    xs = np.zeros((PADN_ALL, D_IN), np.float32)
    xsv = (np.asarray(x, np.float32) * dinv[:, None]).reshape(
        N_CORES, NSHARD, D_IN)
    xs.reshape(N_CORES, PADN, D_IN)[:, :NSHARD] = xsv
    xt = xs.astype(ml_dtypes.bfloat16)

    # per-core dinv*x re-tiled so a group's 4 dst tiles load as one
    # contiguous-per-partition DMA: xg[k, p, t*128+f] = (dinv*x)[k,t*128+p,f]
    xg = np.ascontiguousarray(
        xs.astype(ml_dtypes.bfloat16).reshape(N_CORES, NT, 128, D_IN)
        .transpose(0, 2, 1, 3).reshape(N_CORES, 128, NT * D_IN))

    return xt, xg, dinv_cols, meta


def _build_program(meta, b1_zero, b2_zero):
    f32 = mybir.dt.float32
    bf16 = mybir.dt.bfloat16
    i16 = mybir.dt.int16
    nc = bacc.Bacc("TRN2", target_bir_lowering=False, debug=False,
                   num_devices=N_CORES, num_swdge_queues=4)

    IDXC, CHC = meta["IDXC16"], meta["CHC"]

    xt_in = nc.dram_tensor("xt", [PADN_ALL, D_IN], bf16, kind="ExternalInput").ap()
    xg_in = nc.dram_tensor("xg", [128, NT * D_IN], bf16, kind="ExternalInput").ap()
    idb_in = nc.dram_tensor("identb", [128, 128], bf16, kind="ExternalInput").ap()
    w1_in = nc.dram_tensor("W1b", [D_IN, D_H], bf16, kind="ExternalInput").ap()
    w2_in = nc.dram_tensor("W2b", [D_H, D_OUT], bf16, kind="ExternalInput").ap()
    b1_in = nc.dram_tensor("b1r", [128, D_H], f32, kind="ExternalInput").ap()
    b2_in = nc.dram_tensor("b2r", [128, D_OUT], f32, kind="ExternalInput").ap()
    io_in = nc.dram_tensor("iota", [128, 128], bf16, kind="ExternalInput").ap()
    dv_in = nc.dram_tensor("dinv_cols", [128, NT], f32, kind="ExternalInput").ap()
    ix_in = nc.dram_tensor("idx16", [128, IDXC], i16, kind="ExternalInput").ap()
    dl_in = nc.dram_tensor("dstloc", [128, CHC], bf16, kind="ExternalInput").ap()
    out_t = nc.dram_tensor("out", [PADN, D_OUT], bf16, kind="ExternalOutput").ap()

    rg = [list(range(N_CORES))]

    with tile.TileContext(nc) as tc:
        with tc.tile_pool(name="const", bufs=1) as constp, \
             tc.tile_pool(name="dram", bufs=1, space="DRAM") as dram, \
             tc.tile_pool(name="agg", bufs=4, space="PSUM") as aggp, \
             tc.tile_pool(name="proj", bufs=3, space="PSUM") as projp, \
             tc.tile_pool(name="gat", bufs=6) as gatp, \
             tc.tile_pool(name="sel", bufs=6) as selp, \
             tc.tile_pool(name="dx", bufs=3) as dxp, \
             tc.tile_pool(name="og", bufs=2) as ogp, \
             tc.tile_pool(name="sb", bufs=4) as sb:

            nc.gpsimd.load_library(mlp)

            w1 = constp.tile([D_IN, D_H], bf16)
            nc.sync.dma_start(w1[:], w1_in[:])
            w2 = constp.tile([D_H, D_OUT], bf16)
            nc.sync.dma_start(w2[:], w2_in[:])
            b1r = constp.tile([128, D_H], f32)
            nc.sync.dma_start(b1r[:], b1_in[:])
            b2r = constp.tile([128, D_OUT], f32)
            nc.sync.dma_start(b2r[:], b2_in[:])
            iota = constp.tile([128, 128], bf16)
            nc.sync.dma_start(iota[:], io_in[:])
            identb = constp.tile([128, 128], bf16)
            nc.sync.dma_start(identb[:], idb_in[:])
            dvc = constp.tile([128, NT], f32)
            nc.sync.dma_start(dvc[:], dv_in[:])

            # idx loads in two pieces so the first gathers only wait on the
            # small head chunk; the same metadata serves both layers.
            ixall = constp.tile([128, IDXC], i16)
            ix_head = int(meta["grp_base"][2]) * 8 if N_GROUP > 2 else IDXC
            nc.sync.dma_start(ixall[:, 0:ix_head], ix_in[:, 0:ix_head])
            nc.sync.dma_start(ixall[:, ix_head:IDXC], ix_in[:, ix_head:IDXC])
            dlall = constp.tile([128, CHC], bf16)
            nc.sync.dma_start(dlall[:], dl_in[:])

            # all of t2 stays resident in SBUF for layer-2 self-loops (one
            # pad tile so the [t:t+2] 128-wide self-loop lhsT view is valid
            # for the last tile)
            t2sb = constp.tile([128, NT + 1, D_H], bf16)

            # table2 rows mirror table1 rows: t2 payload in cols 0:64, the
            # rest is junk that lands in unread PSUM partitions 64:128.
            t2_shard = dram.tile([PADN, 128], bf16)
            t2_full = dram.tile([PADN_ALL, 128], bf16, addr_space="Shared")

            def issue_gathers(g, t0g, t1g, tbs, extra_deps=()):
                """4 sub-gathers per group, one per SWDGE queue, each into its
                own G sub-tile so consume releases them independently. Each
                bucket's chunk range splits at a tile boundary so every
                sub-gather still ends in pad slots (trailing-trim safe).
                Returns (subtiles, sub_offsets)."""
                base = int(meta["grp_base"][g])
                chunk_col = meta["chunk_col"]
                grp_b_off = meta["grp_b_off"]
                tmid = (t0g + t1g) // 2
                subs = []
                offs = []
                qn = g
                si = 0
                for b in range(N_BUCKET):
                    lo = int(grp_b_off[g, b])
                    mid = int(chunk_col[tmid, b]) - base
                    hi = int(grp_b_off[g, b + 1])
                    for c0, c1 in ((lo, mid), (mid, hi)):
                        ncb = c1 - c0
                        offs.append(c0)
                        if ncb == 0:
                            subs.append(None)
                            si += 1
                            continue
                        nidx = ncb * 128
                        q = qn % 4
                        G = gatp.tile([128, ncb, D_IN], bf16, tag=f"G{si}")
                        bi = nc.gpsimd.dma_gather(
                            G[:, :, :],
                            tbs[b],
                            ixall[:, (base + c0) * 8:(base + c1) * 8],
                            nidx, nidx, D_IN,
                            single_packet=False,
                            queue_num=q,
                        )
                        for dep in extra_deps:
                            tile.add_dep_helper(
                                bi.ins, dep.ins,
                                reason="AllGather feeds gather table")
                        subs.append(G)
                        si += 1
                        qn += 1
                return subs, offs

            def consume_group(g, t0g, t1g, GA, layer):
                subs, offs = GA
                base = int(meta["grp_base"][g])
                nb = meta["nb"]
                chunk_col = meta["chunk_col"]
                tmid = (t0g + t1g) // 2
                ntl = t1g - t0g
                width = D_IN if layer == 1 else D_H
                if layer == 1:
                    dxt_g = dxp.tile([128, GROUP * D_IN], bf16, tag="dx")
                    nc.sync.dma_start(
                        dxt_g[:, 0:ntl * D_IN],
                        xg_in[:, t0g * D_IN:t1g * D_IN])
                else:
                    og = ogp.tile([128, GROUP, D_OUT], bf16, tag="og")

                # phase 1: all aggregation matmuls of the group (releases the
                # G sub-tiles for the next groups as early as possible)
                aggs = []
                for t in range(t0g, t1g):
                    ti = t - t0g
                    nb0 = int(nb[t, 0])
                    nb1 = int(nb[t, 1])
                    nct = nb0 + nb1
                    l0 = int(chunk_col[t, 0]) - base
                    l1 = int(chunk_col[t, 1]) - base
                    s0 = 0 if t < tmid else 1
                    s1 = 2 if t < tmid else 3
                    S0 = selp.tile([128, nb0, 128], bf16, tag="S0")
                    nc.vector.tensor_tensor(
                        out=S0[:],
                        in0=dlall[:, base + l0:base + l0 + nb0]
                            .to_broadcast([128, nb0, 128]),
                        in1=iota[:].unsqueeze(1).to_broadcast([128, nb0, 128]),
                        op=mybir.AluOpType.is_equal,
                    )
                    S1 = selp.tile([128, nb1, 128], bf16, tag="S1")
                    nc.vector.tensor_tensor(
                        out=S1[:],
                        in0=dlall[:, base + l1:base + l1 + nb1]
                            .to_broadcast([128, nb1, 128]),
                        in1=iota[:].unsqueeze(1).to_broadcast([128, nb1, 128]),
                        op=mybir.AluOpType.is_equal,
                    )
                    # aggT[f, d] = sum_e G[e, f] * S[e, d]  (G stationary
                    # 128-wide -> FWL-hidden LDW, S moving) -> agg arrives
                    # pre-transposed for the projection matmul. In layer 2
                    # feature rows 64:128 accumulate junk and are never read.
                    aggT = aggp.tile([D_IN, 128], f32, tag="agg")
                    if layer == 1:
                        dxt = dxt_g[:, ti * D_IN:(ti + 1) * D_IN]
                    else:
                        dxt = t2sb[:, t:t + 2, :]
                    # self-loop: aggT += dxt.T @ I (dense local rows)
                    nc.tensor.matmul(aggT[:], lhsT=dxt, rhs=identb[:],
                                     start=True, stop=False)
                    for i in range(nct):
                        if i < nb0:
                            l, si = l0 + i, s0
                        else:
                            l, si = l1 + i - nb0, s1
                        S = S0[:, i, :] if i < nb0 else S1[:, i - nb0, :]
                        gcol = l - offs[si]
                        nc.tensor.matmul(
                            aggT[:], lhsT=subs[si][:, gcol, :], rhs=S,
                            start=False, stop=(i == nct - 1),
                        )
                    aggs.append(aggT)

                # phase 2: per-tile projection/activation tails
                for t in range(t0g, t1g):
                    ti = t - t0g
                    aggT = aggs[ti]
                    aggT_sb = sb.tile([width, 128], bf16, tag="e1")
                    nc.scalar.copy(aggT_sb[:], aggT[0:width, :])
                    if layer == 1:
                        # t2 = dinv*relu(dinv*(agg @ W1) + b1)
                        proj = projp.tile([128, D_H], f32, tag="proj")
                        nc.tensor.matmul(proj[:], lhsT=aggT_sb[:], rhs=w1[:],
                                         start=True, stop=True)
                        if b1_zero:
                            hr = sb.tile([128, D_H], f32, tag="e5")
                            nc.scalar.activation(
                                hr[:], proj[:],
                                mybir.ActivationFunctionType.Relu,
                                scale=dvc[:, t:t + 1])
                        else:
                            hv = sb.tile([128, D_H], f32, tag="e3")
                            nc.scalar.activation(
                                hv[:], proj[:],
                                mybir.ActivationFunctionType.Copy,
                                scale=dvc[:, t:t + 1])
                            hb = sb.tile([128, D_H], f32, tag="e4")
                            nc.vector.tensor_add(hb[:], hv[:], b1r[:])
                            hr = sb.tile([128, D_H], f32, tag="e5")
                            nc.scalar.activation(
                                hr[:], hb[:],
                                mybir.ActivationFunctionType.Relu)
                        nc.scalar.activation(
                            t2sb[:, t, :], hr[:],
                            mybir.ActivationFunctionType.Copy,
                            scale=dvc[:, t:t + 1])
                    else:
                        # out = dinv*(agg @ W2) + b2
                        o_ps = projp.tile([128, D_OUT], f32, tag="proj")
                        nc.tensor.matmul(o_ps[:], lhsT=aggT_sb[:], rhs=w2[:],
                                         start=True, stop=True)
                        if b2_zero:
                            nc.scalar.activation(
                                og[:, ti, :], o_ps[:],
                                mybir.ActivationFunctionType.Copy,
                                scale=dvc[:, t:t + 1])
                        else:
                            ov = sb.tile([128, D_OUT], f32, tag="e3")
                            nc.scalar.activation(
                                ov[:], o_ps[:],
                                mybir.ActivationFunctionType.Copy,
                                scale=dvc[:, t:t + 1])
                            nc.vector.tensor_add(og[:, ti, :], ov[:], b2r[:])

                if layer == 1:
                    # t2 write into table2 rows t*128.., payload cols 0:64
                    nc.sync.dma_start(
                        t2_shard[t0g * 128:t1g * 128, 0:D_H]
                        .rearrange("(i p) f -> p i f", i=ntl),
                        t2sb[:, t0g:t1g, :])
                else:
                    nc.sync.dma_start(
                        out_t[t0g * 128:t1g * 128, :]
                        .rearrange("(i p) f -> p i f", i=ntl),
                        og[:, 0:ntl, :])

            tb1 = (xt_in[BIAS[0]:B0_ROWS, :], xt_in[BIAS[1]:PADN_ALL, :])
            tb2 = (t2_full[BIAS[0]:B0_ROWS, :], t2_full[BIAS[1]:PADN_ALL, :])

            # ---- layer 1 (gathers issued three groups ahead of consume) ----
            LA = 3
            gas = [issue_gathers(g, *GROUPS[g], tb1) for g in range(LA)]
            for gp in range(N_GROUP):
                if gp + LA < N_GROUP:
                    gas.append(issue_gathers(gp + LA, *GROUPS[gp + LA], tb1))
                consume_group(gp, *GROUPS[gp], gas[gp], layer=1)

            ag = nc.gpsimd.collective_compute(
                "AllGather", mybir.AluOpType.bypass,
                ins=[t2_shard.opt()], outs=[t2_full.opt()],
                replica_groups=rg,
            )

            # ---- layer 2 ----
            gas = [issue_gathers(g, *GROUPS[g], tb2, extra_deps=(ag,))
                   for g in range(LA)]
            for gp in range(N_GROUP):
                if gp + LA < N_GROUP:
                    gas.append(issue_gathers(gp + LA, *GROUPS[gp + LA], tb2,
                                             extra_deps=(ag,)))
                consume_group(gp, *GROUPS[gp], gas[gp], layer=2)

    nc.compile()
    return nc


def kernel(x, edge_index, W1, b1, W2, b2):
    global LAST_RESULT
    x = np.asarray(x, np.float32)
    W1 = np.asarray(W1, np.float32)
    W2 = np.asarray(W2, np.float32)
    b1 = np.asarray(b1, np.float32)
    b2 = np.asarray(b2, np.float32)

    xt, xg, dinv_cols, meta = _host_prep(x, edge_index)
    nc = _build_program(meta, bool(np.all(b1 == 0.0)),
                        bool(np.all(b2 == 0.0)))

    iota = np.tile(np.arange(128, dtype=np.float32), (128, 1)).astype(
        ml_dtypes.bfloat16)
    identb = np.eye(128, dtype=np.float32).astype(ml_dtypes.bfloat16)
    b1r = np.tile(b1[None, :], (128, 1)).astype(np.float32)
    b2r = np.tile(b2[None, :], (128, 1)).astype(np.float32)
    w1b = W1.astype(ml_dtypes.bfloat16)
    w2b = W2.astype(ml_dtypes.bfloat16)

    in_maps = []
    for k in range(N_CORES):
        in_maps.append({
            "xt": xt, "xg": xg[k], "identb": identb,
            "W1b": w1b, "W2b": w2b, "b1r": b1r, "b2r": b2r,
            "iota": iota,
            "dinv_cols": dinv_cols[k],
            "idx16": meta["idx16"][k],
            "dstloc": meta["dstloc"][k],
        })

    trace = bool(os.environ.get("BASS_TRACE"))
    res = bass_utils.run_bass_kernel_spmd(
        nc, in_maps, core_ids=list(range(N_CORES)), trace=trace)
    LAST_RESULT = res

    out = np.empty((N_NODES, D_OUT), np.float32)
    for k in range(N_CORES):
        out[k * NSHARD:(k + 1) * NSHARD] = np.asarray(
            res.results[k]["out"][:NSHARD], dtype=np.float32)
    return out


# revision 58
# speedup vs baseline: 1.0981x; 1.0981x over previous
"""2-layer GCN on 8 Trainium2 NeuronCores — aggregate-then-project.

Nodes are range-sharded across 8 cores (dst parallel). Both GCN layers are
computed as: gather source rows from a bf16 feature table (dma_gather with
biased signed-int16 indices), segment-sum via one-hot bf16 matmuls into PSUM,
then per-dst-tile projection:

  layer1 table = dinv*x (host-precomputed, full -> no collective needed)
      aggx[d]  = sum_{s->d} table1[s]          (self-loops applied densely)
      t2[d]    = dinv[d] * relu(dinv[d]*(aggx[d] @ W1) + b1)
  AllGather t2 -> table2  [PADN_ALL, 128]: payload in cols 0:64, junk in
      64:128 that only ever reaches unread PSUM partitions -- both layers'
      stationary G operands stay 128-wide (FWL-hidden LDWEIGHTS) and one
      edge-metadata set (idx16/dstloc/one-hot layout) serves both layers.
      out[d]   = dinv[d]*(agg2[d] @ W2) + b2

Per group of 4 dst tiles, 4 sub-gathers land in per-queue G sub-tiles
(independent WAR release); consume runs all aggregation matmuls of the
group first (4 PSUM banks) and the per-tile projection tails afterwards,
so the next group's gathers are released as early as possible.
"""
import os
import sys

sys.path.insert(0, "/opt/trn_rl_repo")

import numpy as np
import ml_dtypes

import concourse.bass as bass
import concourse.bacc as bacc
import concourse.tile as tile
import concourse.mybir as mybir
from concourse import bass_utils
from concourse.library_config import mlp

N_CORES = 8
N_NODES = 100000
D_IN, D_H, D_OUT = 128, 64, 64
NSHARD = N_NODES // N_CORES          # 12500
TILE = 128
NT = (NSHARD + TILE - 1) // TILE     # 98
PADN = NT * TILE                     # 12544
PADN_ALL = N_CORES * PADN            # 100352
B0_ROWS = PADN_ALL // 2              # 50176 bucket split for int16 idx bias
BIAS = (32768, B0_ROWS + 32768)
N_BUCKET = 2
GROUP = 4
GROUPS = [(t, min(t + GROUP, NT)) for t in range(0, NT, GROUP)]
N_GROUP = len(GROUPS)                # 25 (24x4 tiles + 1x2)

LAST_RESULT = None


def _pack_meta(core, t_id, dloc, idx_val_raw, bucket):
    """Slot layout + per-core idx16/dstloc (shared by both edge passes)."""
    key = (core * NT + t_id) * N_BUCKET + bucket
    order = np.argsort(key, kind="stable")
    key_s = key[order]
    idx_s = idx_val_raw[order]
    dloc_s = dloc[order]

    ngroups = N_CORES * NT * N_BUCKET
    counts = np.bincount(key_s, minlength=ngroups).reshape(N_CORES, NT, N_BUCKET)
    nb = -(-counts.max(axis=0) // 128)                  # [NT, N_BUCKET] ceil
    nb = np.maximum(nb, 1)
    # each sub-gather ends at (tmid-1, b) or (t1-1, b); those (t,b) must end
    # with >=1 pad slot on every core (the gather ucode trims trailing
    # negative idxs, which would otherwise drop real edges).
    for (t0, t1) in GROUPS:
        tmid = (t0 + t1) // 2
        for tf in (tmid - 1, t1 - 1):
            for b in range(N_BUCKET):
                if (counts[:, tf, b] == nb[tf, b] * 128).any():
                    nb[tf, b] += 1

    # slot layout: per group of GROUP tiles: all b0 chunks (tile-major), then
    # all b1 chunks -> one contiguous gather dst region per (group, bucket).
    chunk_col = np.zeros((NT, N_BUCKET), np.int64)
    grp_nc = np.zeros(N_GROUP, np.int64)
    grp_base = np.zeros(N_GROUP, np.int64)
    grp_b_off = np.zeros((N_GROUP, N_BUCKET + 1), np.int64)
    pos = 0
    for g, (t0, t1) in enumerate(GROUPS):
        grp_base[g] = pos
        for b in range(N_BUCKET):
            grp_b_off[g, b] = pos - grp_base[g]
            for t in range(t0, t1):
                chunk_col[t, b] = pos
                pos += nb[t, b]
        grp_nc[g] = pos - grp_base[g]
        grp_b_off[g, N_BUCKET] = grp_nc[g]
    CHC = pos
    IDXC16 = CHC * 8

    grp_start = np.zeros(ngroups + 1, np.int64)
    np.cumsum(counts.reshape(-1), out=grp_start[1:])
    rank = np.arange(key_s.shape[0], dtype=np.int64) - grp_start[key_s]

    core_s = key_s // (NT * N_BUCKET)
    tb = key_s % (NT * N_BUCKET)
    t_s = tb // N_BUCKET
    b_s = tb % N_BUCKET

    slot = chunk_col[t_s, b_s] * 128 + rank
    ccol = slot // 128
    cpart = slot % 128

    assert idx_s.min() >= -32768 and idx_s.max() <= 32767

    idx16_16 = np.zeros((N_CORES, 16, IDXC16), np.int16)
    idx16_16[core_s, slot % 16, slot // 16] = idx_s.astype(np.int16)
    idx16 = np.tile(idx16_16, (1, 8, 1))                # [cores, 128, IDXC16]

    dstloc = np.full((N_CORES, 128, CHC), 512.0, np.float32)
    dstloc[core_s, cpart, ccol] = dloc_s.astype(np.float32)
    dstloc = dstloc.astype(ml_dtypes.bfloat16)

    return dict(nb=nb, chunk_col=chunk_col, grp_nc=grp_nc, grp_base=grp_base,
                grp_b_off=grp_b_off, CHC=CHC, IDXC16=IDXC16,
                idx16=idx16, dstloc=dstloc)


def _host_prep(x, edge_index):
    src = np.asarray(edge_index[0], dtype=np.int64)
    dst = np.asarray(edge_index[1], dtype=np.int64)
    n = N_NODES

    deg = np.bincount(dst, minlength=n).astype(np.float64) + 1.0
    dinv = (1.0 / np.sqrt(deg)).astype(np.float32)

    core_d = dst // NSHARD
    drem = dst % NSHARD
    t_id = drem // TILE
    dloc = drem % TILE

    # shared edge metadata: both layers' tables are [PADN_ALL, 128] with row
    # gsrc (padded global node index) -> one idx16/dstloc set serves both.
    gsrc = (src // NSHARD) * PADN + (src % NSHARD)
    bkt = (gsrc >= B0_ROWS).astype(np.int64)
    idxv = gsrc - np.where(bkt == 0, BIAS[0], BIAS[1])
    meta = _pack_meta(core_d, t_id, dloc, idxv, bkt)

    dinv_cols = np.zeros((N_CORES, 128, NT), np.float32)
    node_grid = (
        np.arange(N_CORES)[:, None, None] * NSHARD
        + np.arange(NT)[None, None, :] * TILE
        + np.arange(128)[None, :, None]
    )
    local = np.arange(NT)[None, None, :] * TILE + np.arange(128)[None, :, None]
    valid = np.broadcast_to(local < NSHARD, node_grid.shape)
    dinv_cols[:] = np.where(valid, dinv[np.where(valid, node_grid, 0)], 0.0)

    xs = np.zeros((PADN_ALL, D_IN), np.float32)
    xsv = (np.asarray(x, np.float32) * dinv[:, None]).reshape(
        N_CORES, NSHARD, D_IN)
    xs.reshape(N_CORES, PADN, D_IN)[:, :NSHARD] = xsv
    xt = xs.astype(ml_dtypes.bfloat16)

    # per-core dinv*x re-tiled so a group's 4 dst tiles load as one
    # contiguous-per-partition DMA: xg[k, p, t*128+f] = (dinv*x)[k,t*128+p,f]
    xg = np.ascontiguousarray(
        xs.astype(ml_dtypes.bfloat16).reshape(N_CORES, NT, 128, D_IN)
        .transpose(0, 2, 1, 3).reshape(N_CORES, 128, NT * D_IN))

    return xt, xg, dinv_cols, meta


def _build_program(meta, b1_zero, b2_zero):
    f32 = mybir.dt.float32
    bf16 = mybir.dt.bfloat16
    i16 = mybir.dt.int16
    nc = bacc.Bacc("TRN2", target_bir_lowering=False, debug=False,
                   num_devices=N_CORES, num_swdge_queues=4)

    IDXC, CHC = meta["IDXC16"], meta["CHC"]

    xt_in = nc.dram_tensor("xt", [PADN_ALL, D_IN], bf16, kind="ExternalInput").ap()
    xg_in = nc.dram_tensor("xg", [128, NT * D_IN], bf16, kind="ExternalInput").ap()
    idb_in = nc.dram_tensor("identb", [128, 128], bf16, kind="ExternalInput").ap()
    w1_in = nc.dram_tensor("W1b", [D_IN, D_H], bf16, kind="ExternalInput").ap()
    w2_in = nc.dram_tensor("W2b", [D_H, D_OUT], bf16, kind="ExternalInput").ap()
    b1_in = nc.dram_tensor("b1r", [128, D_H], f32, kind="ExternalInput").ap()
    b2_in = nc.dram_tensor("b2r", [128, D_OUT], f32, kind="ExternalInput").ap()
    io_in = nc.dram_tensor("iota", [128, 128], bf16, kind="ExternalInput").ap()
    dv_in = nc.dram_tensor("dinv_cols", [128, NT], f32, kind="ExternalInput").ap()
    ix_in = nc.dram_tensor("idx16", [128, IDXC], i16, kind="ExternalInput").ap()
    dl_in = nc.dram_tensor("dstloc", [128, CHC], bf16, kind="ExternalInput").ap()
    out_t = nc.dram_tensor("out", [PADN, D_OUT], bf16, kind="ExternalOutput").ap()

    rg = [list(range(N_CORES))]

    with tile.TileContext(nc) as tc:
        with tc.tile_pool(name="const", bufs=1) as constp, \
             tc.tile_pool(name="dram", bufs=1, space="DRAM") as dram, \
             tc.tile_pool(name="agg", bufs=4, space="PSUM") as aggp, \
             tc.tile_pool(name="proj", bufs=3, space="PSUM") as projp, \
             tc.tile_pool(name="gat", bufs=6) as gatp, \
             tc.tile_pool(name="sel", bufs=6) as selp, \
             tc.tile_pool(name="dx", bufs=3) as dxp, \
             tc.tile_pool(name="og", bufs=2) as ogp, \
             tc.tile_pool(name="sb", bufs=4) as sb:

            nc.gpsimd.load_library(mlp)

            w1 = constp.tile([D_IN, D_H], bf16)
            nc.sync.dma_start(w1[:], w1_in[:])
            w2 = constp.tile([D_H, D_OUT], bf16)
            nc.sync.dma_start(w2[:], w2_in[:])
            b1r = constp.tile([128, D_H], f32)
            nc.sync.dma_start(b1r[:], b1_in[:])
            b2r = constp.tile([128, D_OUT], f32)
            nc.sync.dma_start(b2r[:], b2_in[:])
            iota = constp.tile([128, 128], bf16)
            nc.sync.dma_start(iota[:], io_in[:])
            identb = constp.tile([128, 128], bf16)
            nc.sync.dma_start(identb[:], idb_in[:])
            dvc = constp.tile([128, NT], f32)
            nc.sync.dma_start(dvc[:], dv_in[:])

            # idx loads in two pieces so the first gathers only wait on the
            # small head chunk; the same metadata serves both layers.
            ixall = constp.tile([128, IDXC], i16)
            ix_head = int(meta["grp_base"][2]) * 8 if N_GROUP > 2 else IDXC
            nc.sync.dma_start(ixall[:, 0:ix_head], ix_in[:, 0:ix_head])
            nc.sync.dma_start(ixall[:, ix_head:IDXC], ix_in[:, ix_head:IDXC])
            dlall = constp.tile([128, CHC], bf16)
            nc.sync.dma_start(dlall[:], dl_in[:])

            # all of t2 stays resident in SBUF for layer-2 self-loops (one
            # pad tile so the [t:t+2] 128-wide self-loop lhsT view is valid
            # for the last tile)
            t2sb = constp.tile([128, NT + 1, D_H], bf16)

            # table2 rows mirror table1 rows: t2 payload in cols 0:64, the
            # rest is junk that lands in unread PSUM partitions 64:128.
            t2_shard = dram.tile([PADN, 128], bf16)
            t2_full = dram.tile([PADN_ALL, 128], bf16, addr_space="Shared")

            def issue_gathers(g, t0g, t1g, tbs, extra_deps=()):
                """4 sub-gathers per group, one per SWDGE queue, each into its
                own G sub-tile so consume releases them independently. Each
                bucket's chunk range splits at a tile boundary so every
                sub-gather still ends in pad slots (trailing-trim safe).
                Returns (subtiles, sub_offsets)."""
                base = int(meta["grp_base"][g])
                chunk_col = meta["chunk_col"]
                grp_b_off = meta["grp_b_off"]
                tmid = (t0g + t1g) // 2
                subs = []
                offs = []
                qn = g
                si = 0
                for b in range(N_BUCKET):
                    lo = int(grp_b_off[g, b])
                    mid = int(chunk_col[tmid, b]) - base
                    hi = int(grp_b_off[g, b + 1])
                    for c0, c1 in ((lo, mid), (mid, hi)):
                        ncb = c1 - c0
                        offs.append(c0)
                        if ncb == 0:
                            subs.append(None)
                            si += 1
                            continue
                        nidx = ncb * 128
                        q = qn % 4
                        G = gatp.tile([128, ncb, D_IN], bf16, tag=f"G{si}")
                        bi = nc.gpsimd.dma_gather(
                            G[:, :, :],
                            tbs[b],
                            ixall[:, (base + c0) * 8:(base + c1) * 8],
                            nidx, nidx, D_IN,
                            single_packet=False,
                            queue_num=q,
                        )
                        for dep in extra_deps:
                            tile.add_dep_helper(
                                bi.ins, dep.ins,
                                reason="AllGather feeds gather table")
                        subs.append(G)
                        si += 1
                        qn += 1
                return subs, offs

            def consume_group(g, t0g, t1g, GA, layer):
                subs, offs = GA
                base = int(meta["grp_base"][g])
                nb = meta["nb"]
                chunk_col = meta["chunk_col"]
                tmid = (t0g + t1g) // 2
                ntl = t1g - t0g
                width = D_IN if layer == 1 else D_H
                if layer == 1:
                    dxt_g = dxp.tile([128, GROUP * D_IN], bf16, tag="dx")
                    nc.sync.dma_start(
                        dxt_g[:, 0:ntl * D_IN],
                        xg_in[:, t0g * D_IN:t1g * D_IN])
                else:
                    og = ogp.tile([128, GROUP, D_OUT], bf16, tag="og")

                # phase 1: all aggregation matmuls of the group (releases the
                # G sub-tiles for the next groups as early as possible)
                aggs = []
                for t in range(t0g, t1g):
                    ti = t - t0g
                    nb0 = int(nb[t, 0])
                    nb1 = int(nb[t, 1])
                    nct = nb0 + nb1
                    l0 = int(chunk_col[t, 0]) - base
                    l1 = int(chunk_col[t, 1]) - base
                    s0 = 0 if t < tmid else 1
                    s1 = 2 if t < tmid else 3
                    S0 = selp.tile([128, nb0, 128], bf16, tag="S0")
                    nc.vector.tensor_tensor(
                        out=S0[:],
                        in0=dlall[:, base + l0:base + l0 + nb0]
                            .to_broadcast([128, nb0, 128]),
                        in1=iota[:].unsqueeze(1).to_broadcast([128, nb0, 128]),
                        op=mybir.AluOpType.is_equal,
                    )
                    S1 = selp.tile([128, nb1, 128], bf16, tag="S1")
                    nc.vector.tensor_tensor(
                        out=S1[:],
                        in0=dlall[:, base + l1:base + l1 + nb1]
                            .to_broadcast([128, nb1, 128]),
                        in1=iota[:].unsqueeze(1).to_broadcast([128, nb1, 128]),
                        op=mybir.AluOpType.is_equal,
                    )
                    # aggT[f, d] = sum_e G[e, f] * S[e, d]  (G stationary
                    # 128-wide -> FWL-hidden LDW, S moving) -> agg arrives
                    # pre-transposed for the projection matmul. In layer 2
                    # feature rows 64:128 accumulate junk and are never read.
                    aggT = aggp.tile([D_IN, 128], f32, tag="agg")
                    if layer == 1:
                        dxt = dxt_g[:, ti * D_IN:(ti + 1) * D_IN]
                    else:
                        dxt = t2sb[:, t:t + 2, :]
                    # self-loop: aggT += dxt.T @ I (dense local rows)
                    nc.tensor.matmul(aggT[:], lhsT=dxt, rhs=identb[:],
                                     start=True, stop=False)
                    for i in range(nct):
                        if i < nb0:
                            l, si = l0 + i, s0
                        else:
                            l, si = l1 + i - nb0, s1
                        S = S0[:, i, :] if i < nb0 else S1[:, i - nb0, :]
                        gcol = l - offs[si]
                        nc.tensor.matmul(
                            aggT[:], lhsT=subs[si][:, gcol, :], rhs=S,
                            start=False, stop=(i == nct - 1),
                        )
                    aggs.append(aggT)

                # phase 2: per-tile projection/activation tails
                for t in range(t0g, t1g):
                    ti = t - t0g
                    aggT = aggs[ti]
                    aggT_sb = sb.tile([width, 128], bf16, tag="e1")
                    nc.scalar.copy(aggT_sb[:], aggT[0:width, :])
                    if layer == 1:
                        # t2 = dinv*relu(dinv*(agg @ W1) + b1)
                        proj = projp.tile([128, D_H], f32, tag="proj")
                        nc.tensor.matmul(proj[:], lhsT=aggT_sb[:], rhs=w1[:],
                                         start=True, stop=True)
                        if b1_zero:
                            hr = sb.tile([128, D_H], f32, tag="e5")
                            nc.scalar.activation(
                                hr[:], proj[:],
                                mybir.ActivationFunctionType.Relu,
                                scale=dvc[:, t:t + 1])
                        else:
                            hv = sb.tile([128, D_H], f32, tag="e3")
                            nc.scalar.activation(
                                hv[:], proj[:],
                                mybir.ActivationFunctionType.Copy,
                                scale=dvc[:, t:t + 1])
                            hb = sb.tile([128, D_H], f32, tag="e4")
                            nc.vector.tensor_add(hb[:], hv[:], b1r[:])
                            hr = sb.tile([128, D_H], f32, tag="e5")
                            nc.scalar.activation(
                                hr[:], hb[:],
                                mybir.ActivationFunctionType.Relu)
                        nc.scalar.activation(
                            t2sb[:, t, :], hr[:],
                            mybir.ActivationFunctionType.Copy,
                            scale=dvc[:, t:t + 1])
                    else:
                        # out = dinv*(agg @ W2) + b2
                        o_ps = projp.tile([128, D_OUT], f32, tag="proj")
                        nc.tensor.matmul(o_ps[:], lhsT=aggT_sb[:], rhs=w2[:],
                                         start=True, stop=True)
                        if b2_zero:
                            nc.scalar.activation(
                                og[:, ti, :], o_ps[:],
                                mybir.ActivationFunctionType.Copy,
                                scale=dvc[:, t:t + 1])
                        else:
                            ov = sb.tile([128, D_OUT], f32, tag="e3")
                            nc.scalar.activation(
                                ov[:], o_ps[:],
                                mybir.ActivationFunctionType.Copy,
                                scale=dvc[:, t:t + 1])
                            nc.vector.tensor_add(og[:, ti, :], ov[:], b2r[:])

                if layer == 1:
                    # t2 write into table2 rows t*128.., payload cols 0:64
                    nc.sync.dma_start(
                        t2_shard[t0g * 128:t1g * 128, 0:D_H]
                        .rearrange("(i p) f -> p i f", i=ntl),
                        t2sb[:, t0g:t1g, :])
                else:
                    nc.sync.dma_start(
                        out_t[t0g * 128:t1g * 128, :]
                        .rearrange("(i p) f -> p i f", i=ntl),
                        og[:, 0:ntl, :])

            tb1 = (xt_in[BIAS[0]:B0_ROWS, :], xt_in[BIAS[1]:PADN_ALL, :])
            tb2 = (t2_full[BIAS[0]:B0_ROWS, :], t2_full[BIAS[1]:PADN_ALL, :])

            # ---- layer 1 (gathers issued two groups ahead of consume) ----
            gas = [issue_gathers(g, *GROUPS[g], tb1) for g in range(2)]
            for gp in range(N_GROUP):
                if gp + 2 < N_GROUP:
                    gas.append(issue_gathers(gp + 2, *GROUPS[gp + 2], tb1))
                consume_group(gp, *GROUPS[gp], gas[gp], layer=1)

            ag = nc.gpsimd.collective_compute(
                "AllGather", mybir.AluOpType.bypass,
                ins=[t2_shard.opt()], outs=[t2_full.opt()],
                replica_groups=rg,
            )

            # ---- layer 2 ----
            gas = [issue_gathers(g, *GROUPS[g], tb2, extra_deps=(ag,))
                   for g in range(2)]
            for gp in range(N_GROUP):
                if gp + 2 < N_GROUP:
                    gas.append(issue_gathers(gp + 2, *GROUPS[gp + 2], tb2,
                                             extra_deps=(ag,)))
                consume_group(gp, *GROUPS[gp], gas[gp], layer=2)

    nc.compile()
    return nc


def kernel(x, edge_index, W1, b1, W2, b2):
    global LAST_RESULT
    x = np.asarray(x, np.float32)
    W1 = np.asarray(W1, np.float32)
    W2 = np.asarray(W2, np.float32)
    b1 = np.asarray(b1, np.float32)
    b2 = np.asarray(b2, np.float32)

    xt, xg, dinv_cols, meta = _host_prep(x, edge_index)
    nc = _build_program(meta, bool(np.all(b1 == 0.0)),
                        bool(np.all(b2 == 0.0)))

    iota = np.tile(np.arange(128, dtype=np.float32), (128, 1)).astype(
        ml_dtypes.bfloat16)
    identb = np.eye(128, dtype=np.float32).astype(ml_dtypes.bfloat16)
    b1r = np.tile(b1[None, :], (128, 1)).astype(np.float32)
    b2r = np.tile(b2[None, :], (128, 1)).astype(np.float32)
    w1b = W1.astype(ml_dtypes.bfloat16)
    w2b = W2.astype(ml_dtypes.bfloat16)

    in_maps = []
    for k in range(N_CORES):
        in_maps.append({
            "xt": xt, "xg": xg[k], "identb": identb,
            "W1b": w1b, "W2b": w2b, "b1r": b1r, "b2r": b2r,
            "iota": iota,
            "dinv_cols": dinv_cols[k],
            "idx16": meta["idx16"][k],
            "dstloc": meta["dstloc"][k],
        })

    trace = bool(os.environ.get("BASS_TRACE"))
    res = bass_utils.run_bass_kernel_spmd(
        nc, in_maps, core_ids=list(range(N_CORES)), trace=trace)
    LAST_RESULT = res

    out = np.empty((N_NODES, D_OUT), np.float32)
    for k in range(N_CORES):
        out[k * NSHARD:(k + 1) * NSHARD] = np.asarray(
            res.results[k]["out"][:NSHARD], dtype=np.float32)
    return out
